# revision 19
# baseline (speedup 1.0000x reference)
"""Trainium2 Bass kernel for a SAGAN-style 2D attention layer.

Key-split (split-KV) variant: 8 cores = 4 batches x 2 KEY-halves.
Each core computes, for ALL 4096 queries of its batch, the partial
attention over ITS 2048 keys:

    po = exp(s_half - C_SHIFT) @ h_half     # [4096, 512]
    r  = rowsum(exp(s_half - C_SHIFT))      # [4096]

The host combines the two halves exactly (the fixed shift makes the
partial sums directly addable -- no per-row max renormalization):

    out = gamma * (poA + poB) / (rA + rB) + xf + gamma*bh

vs the query-split version this halves the per-core h projection
(values are only needed for the core's own keys) and shrinks phase A;
s / exp / o work per core is identical.

Other performance notes are inherited from the query-split version:
fp8 everywhere, DoubleRow o/h/f/g matmuls, per-512-token chunked
partition-major-contiguous DMAs, s-pairs row-tiled into PE quadrants
and woven 1-per-4-j into the o-loop so the scalar engine's EXP rate
(1114ns/pair) never throttles the PE, h-eviction casts split
vector/scalar. C_SHIFT = 104.5 per the host-simulated fp8 max logit
113.28 (8.8x fp8e5 overflow margin; softmax is shift-invariant so any
shift error cancels in the division, rows fully below the fp8e5
window flush to zero and are caught by the host's rowsum clamp).
"""

import ml_dtypes
import numpy as np
from contextlib import ExitStack

import concourse.bass as bass
import concourse.mybir as mybir
import concourse.tile as tile
from concourse import bacc, bass_utils

P = 128          # partitions
N = 4096         # tokens per batch (64*64)
NK = 2048        # keys per core (half)
C = 512          # channels
CF = 64          # f/g channels
KC = C // P      # contraction chunks over channels (4)
NJB = NK // P    # 16 local key blocks
NSUP = N // C    # 8 query super-blocks of 512
NQT = N // P     # 32 query blocks of 128
NT = N // C      # 8 token chunks of 512
NKT = NK // C    # 4 key-token chunks
C_SHIFT = 104.5

f32 = mybir.dt.float32
f8e4 = mybir.dt.float8e4
f8e5 = mybir.dt.float8e5

AFT = mybir.ActivationFunctionType
OP = mybir.AluOpType
DR = mybir.MatmulPerfMode.DoubleRow

_PROGRAM = None
LAST_RESULTS = None


def _build_program() -> bass.Bass:
    nc = bacc.Bacc("TRN2", target_bir_lowering=False, debug=False,
                   num_devices=8)

    xT = nc.dram_tensor("xT", [P, NT, KC, C], f8e4,
                        kind="ExternalInput").ap()
    wf = nc.dram_tensor("wf", [P, KC, CF], f8e4, kind="ExternalInput").ap()
    wg = nc.dram_tensor("wg", [P, KC, CF], f8e4, kind="ExternalInput").ap()
    wh = nc.dram_tensor("wh", [P, KC, C], f8e4, kind="ExternalInput").ap()
    bfv = nc.dram_tensor("bfv", [CF, 1], f32, kind="ExternalInput").ap()
    bgv = nc.dram_tensor("bgv", [CF, 1], f32, kind="ExternalInput").ap()
    po_out = nc.dram_tensor("po", [N, C], f32, kind="ExternalOutput").ap()
    r_out = nc.dram_tensor("r", [P, NQT], f32, kind="ExternalOutput").ap()

    with tile.TileContext(nc) as tc, ExitStack() as ctx:
        persist = ctx.enter_context(tc.tile_pool(name="persist", bufs=1))
        fin = ctx.enter_context(tc.tile_pool(name="fin", bufs=3))
        expp = ctx.enter_context(tc.tile_pool(name="expp", bufs=2))
        psS = ctx.enter_context(tc.tile_pool(name="psS", bufs=2, space="PSUM"))

        xt = []
        for tc_i in range(NT):
            xt.append(persist.tile([P, KC, C], f8e4, name=f"xt{tc_i}"))
        nc.sync.dma_start(xt[0], xT[:, 0, :, :])

        wf_sb = persist.tile([P, KC, CF], f8e4)
        nc.gpsimd.dma_start(wf_sb, wf)
        wg_sb = persist.tile([P, KC, CF], f8e4)
        nc.gpsimd.dma_start(wg_sb, wg)
        bf_sb = persist.tile([CF, 1], f32)
        nc.gpsimd.dma_start(bf_sb, bfv)
        bg_sb = persist.tile([CF, 1], f32)
        nc.gpsimd.dma_start(bg_sb, bgv)
        neg_shift = persist.tile([P, 1], f32)
        nc.vector.memset(neg_shift, -C_SHIFT)
        ones2 = persist.tile([P, 2, 1], f8e4)
        nc.vector.memset(ones2, 1.0)

        for tc_i in range(1, NT):
            nc.sync.dma_start(xt[tc_i], xT[:, tc_i, :, :])

        wh_sb = persist.tile([P, KC, C], f8e4)
        nc.sync.dma_start(wh_sb, wh)

        h_sb = persist.tile([P, NJB, C], f8e4)      # values, own keys
        f_sb = persist.tile([P, NK], f8e4)          # f^T own keys + dup
        g_sb = persist.tile([P, N], f8e4)           # g^T all queries + dup
        r_sb = persist.tile([P, NQT], f32)          # rowsums per q-block

        expT_tiles = {}
        spair_queues = {}

        def prep_s_exp(sup):
            expT = expp.tile([P, C // P, NJB, P], f8e5, tag="expT",
                             name=f"expT{sup}")
            expT_tiles[sup] = expT

            def mk(jc2):
                def emit():
                    jc = 2 * jc2
                    ps = psS.tile([P, 2, C], f32, tag="ps",
                                  name=f"ps{sup}_{jc2}")
                    nc.tensor.matmul(ps[:, 0, :],
                                     f_sb[:CF, jc * P:(jc + 1) * P],
                                     g_sb[:CF, sup * C:(sup + 1) * C],
                                     start=True, stop=True,
                                     tile_position=(0, 0))
                    nc.tensor.matmul(ps[:, 1, :],
                                     f_sb[CF:, (jc + 1) * P:(jc + 2) * P],
                                     g_sb[CF:, sup * C:(sup + 1) * C],
                                     start=True, stop=True,
                                     tile_position=(64, 0))
                    nc.scalar.activation(
                        expT[:, :, jc:jc + 2, :],
                        ps.rearrange("p two (qb col) -> p qb two col",
                                     qb=C // P),
                        AFT.Exp, bias=neg_shift)
                return emit
            spair_queues[sup] = [mk(j) for j in range(NJB // 2)]

        # ---- Phase A ----
        with tc.tile_pool(name="psA", bufs=4, space="PSUM") as psA:

            def proj_fg(tc_i, w_sb, b_sb, dst, tag):
                pa = psA.tile([P, C], f32, tag="pa", name=f"p{tag}{tc_i}")
                pp = pa[:CF, :]
                for i2 in range(KC // 2):
                    nc.tensor.matmul(pp, w_sb[:, 2 * i2:2 * i2 + 2, :],
                                     xt[tc_i][:, 2 * i2:2 * i2 + 2, :],
                                     start=(i2 == 0), stop=(i2 == KC // 2 - 1),
                                     perf_mode=DR)
                sl = slice(tc_i * C, (tc_i + 1) * C)
                nc.vector.tensor_scalar_add(dst[:CF, sl], pp, b_sb)
                nc.gpsimd.dma_start(dst[CF:, sl], dst[:CF, sl])

            prep_s_exp(0)
            s0 = spair_queues[0]
            ns0 = 0

            # s0 pair jc2 uses f chunk jc2//2 and g chunk 0 -- woven
            # once their dup DMAs have landed
            for tc_i in range(NT):
                if tc_i < NKT:
                    proj_fg(tc_i, wf_sb, bf_sb, f_sb, "f")
                proj_fg(tc_i, wg_sb, bg_sb, g_sb, "g")
                if 2 <= tc_i < 2 + NJB // 4:     # pairs 0..3 here
                    s0[ns0]()
                    ns0 += 1

            for jb in range(NJB):
                tc_i, jl = divmod(jb, KC)
                ph = psA.tile([P, C], f32, tag="pa")
                for i2 in range(KC // 2):
                    nc.tensor.matmul(ph,
                                     xt[tc_i][:, 2 * i2:2 * i2 + 2,
                                              jl * P:(jl + 1) * P],
                                     wh_sb[:, 2 * i2:2 * i2 + 2, :],
                                     start=(i2 == 0), stop=(i2 == KC // 2 - 1),
                                     perf_mode=DR)
                if jb % 2 == 0:
                    nc.scalar.activation(h_sb[:, jb, :], ph, AFT.Copy)
                else:
                    nc.vector.tensor_copy(h_sb[:, jb, :], ph)
                # front-load the remaining s0 pairs (jb 1,3,5,7) so the
                # last exp0 lands well before phase B's first o-matmul
                if jb % 2 == 1 and ns0 < len(s0):
                    s0[ns0]()
                    ns0 += 1
            while ns0 < len(s0):
                s0[ns0]()
                ns0 += 1

        # ---- Phase B ----
        with tc.tile_pool(name="psO", bufs=2, space="PSUM") as psO, \
             tc.tile_pool(name="psR", bufs=2, space="PSUM") as psR:

            for sup in range(NSUP):
                if sup + 1 < NSUP:
                    prep_s_exp(sup + 1)
                snext = spair_queues.get(sup + 1, [])
                expT = expT_tiles.pop(sup)
                for q in range(C // P):
                    iq = sup * (C // P) + q
                    po = psO.tile([P, C], f32, tag="po")
                    pr = psR.tile([P, 1], f32, tag="pr")
                    for j in range(NJB // 2):
                        lhs = expT[:, q, 2 * j:2 * j + 2, :]
                        nc.tensor.matmul(po, lhs, h_sb[:, 2 * j:2 * j + 2, :],
                                         start=(j == 0),
                                         stop=(j == NJB // 2 - 1),
                                         perf_mode=DR)
                        nc.tensor.matmul(pr, lhs, ones2,
                                         start=(j == 0),
                                         stop=(j == NJB // 2 - 1),
                                         perf_mode=DR)
                        if j % 4 == 1:
                            slot = q * 2 + j // 4
                            if slot < len(snext):
                                snext[slot]()
                    nc.vector.tensor_copy(r_sb[:, iq:iq + 1], pr)
                    ot = fin.tile([P, C], f32, tag="ot")
                    if iq == NQT - 1:
                        # final store in halves: the first half's DMA
                        # overlaps the second half's eviction (tail)
                        for hf in range(2):
                            sl = slice(hf * (C // 2), (hf + 1) * (C // 2))
                            nc.vector.tensor_copy(ot[:, sl], po[:, sl])
                            nc.sync.dma_start(
                                po_out[iq * P:(iq + 1) * P, sl], ot[:, sl])
                    else:
                        nc.vector.tensor_copy(ot, po)
                        nc.sync.dma_start(po_out[iq * P:(iq + 1) * P, :], ot)
                # rowsums trickle out once per superblock from gpsimd
                nc.gpsimd.dma_start(
                    r_out[:, sup * (C // P):(sup + 1) * (C // P)],
                    r_sb[:, sup * (C // P):(sup + 1) * (C // P)])

    nc.compile()
    return nc


def _get_program() -> bass.Bass:
    global _PROGRAM
    if _PROGRAM is None:
        _PROGRAM = _build_program()
    return _PROGRAM


def kernel(x, kernel_f, kernel_g, kernel_h, bias_f, bias_g, bias_h, gamma,
           _trace=False, _trace_kwargs=None):
    global LAST_RESULTS
    x = np.asarray(x, np.float32)
    B = x.shape[0]
    xf = np.ascontiguousarray(x.reshape(B, N, C))
    gamma_f = np.asarray(gamma, np.float32).reshape(())

    e4 = ml_dtypes.float8_e4m3

    def pmajor(w, cout):
        w = np.asarray(w, np.float32).astype(e4)
        return np.ascontiguousarray(w.reshape(KC, P, cout).transpose(1, 0, 2))

    wf_np = pmajor(kernel_f, CF)
    wg_np = pmajor(kernel_g, CF)
    wh_np = pmajor(kernel_h, C)
    bf_np = np.ascontiguousarray(np.asarray(bias_f, np.float32).reshape(CF, 1))
    bg_np = np.ascontiguousarray(np.asarray(bias_g, np.float32).reshape(CF, 1))

    in_maps = []
    for c in range(8):
        b, half = divmod(c, 2)
        xT_full = xf[b].T                       # [C, N]
        if half == 0:
            xT_c = xT_full
        else:
            # rotate this core's key half to the front; g (queries) then
            # runs in rotated order and the host unrotates its po rows
            xT_c = np.concatenate([xT_full[:, NK:], xT_full[:, :NK]], axis=1)
        xT_c = xT_c.astype(e4).reshape(KC, P, NT, C).transpose(1, 2, 0, 3)
        in_maps.append({
            "xT": np.ascontiguousarray(xT_c),
            "wf": wf_np, "wg": wg_np, "wh": wh_np,
            "bfv": bf_np, "bgv": bg_np,
        })

    nc = _get_program()
    LAST_RESULTS = bass_utils.run_bass_kernel_spmd(
        nc, in_maps, core_ids=list(range(8)),
        trace=_trace, **(_trace_kwargs or {}))

    # host combine: exact flash-attention split-KV reduction (fixed
    # shift => partial numerators/denominators are directly addable);
    # h bias folded via softmax row-sum-1: beta@(h0+1*bh) = beta@h0+bh
    res_bias = (gamma_f * np.asarray(bias_h, np.float32)).reshape(1, C)
    result = np.empty((B, N, C), np.float32)
    for b in range(B):
        rs = LAST_RESULTS.results[2 * b]
        po = np.asarray(rs["po"])
        r = np.asarray(rs["r"]).T.reshape(N)
        rs2 = LAST_RESULTS.results[2 * b + 1]
        po2 = np.asarray(rs2["po"])
        r2 = np.asarray(rs2["r"]).T.reshape(N)
        # core 2b+1 ran queries in rotated token order
        po2 = np.concatenate([po2[NK:], po2[:NK]], axis=0)
        r2 = np.concatenate([r2[NK:], r2[:NK]], axis=0)
        o = (po + po2) / np.maximum(r + r2, 1e-30)[:, None]
        result[b] = gamma_f * o + xf[b] + res_bias
    return result.reshape(x.shape)


# revision 20
# speedup vs baseline: 1.0237x; 1.0237x over previous
"""Trainium2 Bass kernel for a SAGAN-style 2D attention layer.

Key-split (split-KV) variant: 8 cores = 4 batches x 2 KEY-halves.
Each core computes, for ALL 4096 queries of its batch, the partial
attention over ITS 2048 keys:

    po = exp(s_half - C_SHIFT) @ h_half     # [4096, 512]
    r  = rowsum(exp(s_half - C_SHIFT))      # [4096]

The host combines the two halves exactly (the fixed shift makes the
partial sums directly addable -- no per-row max renormalization):

    out = gamma * (poA + poB) / (rA + rB) + xf + gamma*bh

vs the query-split version this halves the per-core h projection
(values are only needed for the core's own keys) and shrinks phase A;
s / exp / o work per core is identical.

Other performance notes are inherited from the query-split version:
fp8 everywhere, DoubleRow o/h/f/g matmuls, per-512-token chunked
partition-major-contiguous DMAs, s-pairs row-tiled into PE quadrants
and woven 1-per-4-j into the o-loop so the scalar engine's EXP rate
(1114ns/pair) never throttles the PE, h-eviction casts split
vector/scalar. C_SHIFT = 104.5 per the host-simulated fp8 max logit
113.28 (8.8x fp8e5 overflow margin; softmax is shift-invariant so any
shift error cancels in the division, rows fully below the fp8e5
window flush to zero and are caught by the host's rowsum clamp).
"""

import ml_dtypes
import numpy as np
from contextlib import ExitStack

import concourse.bass as bass
import concourse.mybir as mybir
import concourse.tile as tile
from concourse import bacc, bass_utils

P = 128          # partitions
N = 4096         # tokens per batch (64*64)
NK = 2048        # keys per core (half)
C = 512          # channels
CF = 64          # f/g channels
KC = C // P      # contraction chunks over channels (4)
NJB = NK // P    # 16 local key blocks
NSUP = N // C    # 8 query super-blocks of 512
NQT = N // P     # 32 query blocks of 128
NT = N // C      # 8 token chunks of 512
NKT = NK // C    # 4 key-token chunks
C_SHIFT = 104.5

f32 = mybir.dt.float32
f8e4 = mybir.dt.float8e4
f8e5 = mybir.dt.float8e5

AFT = mybir.ActivationFunctionType
OP = mybir.AluOpType
DR = mybir.MatmulPerfMode.DoubleRow

_PROGRAM = None
LAST_RESULTS = None


def _build_program() -> bass.Bass:
    nc = bacc.Bacc("TRN2", target_bir_lowering=False, debug=False,
                   num_devices=8)

    xT = nc.dram_tensor("xT", [P, NT, KC, C], f8e4,
                        kind="ExternalInput").ap()
    wf = nc.dram_tensor("wf", [P, KC, CF], f8e4, kind="ExternalInput").ap()
    wg = nc.dram_tensor("wg", [P, KC, CF], f8e4, kind="ExternalInput").ap()
    wh = nc.dram_tensor("wh", [P, KC, C], f8e4, kind="ExternalInput").ap()
    bfv = nc.dram_tensor("bfv", [CF, 1], f32, kind="ExternalInput").ap()
    bgv = nc.dram_tensor("bgv", [CF, 1], f32, kind="ExternalInput").ap()
    po_out = nc.dram_tensor("po", [N, C], f32, kind="ExternalOutput").ap()
    r_out = nc.dram_tensor("r", [P, NQT], f32, kind="ExternalOutput").ap()

    with tile.TileContext(nc) as tc, ExitStack() as ctx:
        persist = ctx.enter_context(tc.tile_pool(name="persist", bufs=1))
        fin = ctx.enter_context(tc.tile_pool(name="fin", bufs=3))
        expp = ctx.enter_context(tc.tile_pool(name="expp", bufs=2))
        psS = ctx.enter_context(tc.tile_pool(name="psS", bufs=2, space="PSUM"))

        xt = []
        for tc_i in range(NT):
            xt.append(persist.tile([P, KC, C], f8e4, name=f"xt{tc_i}"))
        nc.sync.dma_start(xt[0], xT[:, 0, :, :])

        wf_sb = persist.tile([P, KC, CF], f8e4)
        nc.gpsimd.dma_start(wf_sb, wf)
        wg_sb = persist.tile([P, KC, CF], f8e4)
        nc.gpsimd.dma_start(wg_sb, wg)
        bf_sb = persist.tile([CF, 1], f32)
        nc.gpsimd.dma_start(bf_sb, bfv)
        bg_sb = persist.tile([CF, 1], f32)
        nc.gpsimd.dma_start(bg_sb, bgv)
        neg_shift = persist.tile([P, 1], f32)
        nc.vector.memset(neg_shift, -C_SHIFT)
        ones2 = persist.tile([P, 2, 1], f8e4)
        nc.vector.memset(ones2, 1.0)

        for tc_i in range(1, NT):
            nc.sync.dma_start(xt[tc_i], xT[:, tc_i, :, :])

        wh_sb = persist.tile([P, KC, C], f8e4)
        nc.sync.dma_start(wh_sb, wh)

        h_sb = persist.tile([P, NJB, C], f8e4)      # values, own keys
        f_sb = persist.tile([P, NK], f8e4)          # f^T own keys + dup
        g_sb = persist.tile([P, N], f8e4)           # g^T all queries + dup
        r_sb = persist.tile([P, NQT], f32)          # rowsums per q-block

        expT_tiles = {}
        spair_queues = {}

        def prep_s_exp(sup):
            expT = expp.tile([P, C // P, NJB, P], f8e5, tag="expT",
                             name=f"expT{sup}")
            expT_tiles[sup] = expT

            def mk(jc2):
                def emit():
                    jc = 2 * jc2
                    ps = psS.tile([P, 2, C], f32, tag="ps",
                                  name=f"ps{sup}_{jc2}")
                    nc.tensor.matmul(ps[:, 0, :],
                                     f_sb[:CF, jc * P:(jc + 1) * P],
                                     g_sb[:CF, sup * C:(sup + 1) * C],
                                     start=True, stop=True,
                                     tile_position=(0, 0))
                    nc.tensor.matmul(ps[:, 1, :],
                                     f_sb[CF:, (jc + 1) * P:(jc + 2) * P],
                                     g_sb[CF:, sup * C:(sup + 1) * C],
                                     start=True, stop=True,
                                     tile_position=(64, 0))
                    nc.scalar.activation(
                        expT[:, :, jc:jc + 2, :],
                        ps.rearrange("p two (qb col) -> p qb two col",
                                     qb=C // P),
                        AFT.Exp, bias=neg_shift)
                return emit
            spair_queues[sup] = [mk(j) for j in range(NJB // 2)]

        # ---- Phase A ----
        with tc.tile_pool(name="psA", bufs=4, space="PSUM") as psA:

            def proj_fg(tc_i, w_sb, b_sb, dst, tag):
                pa = psA.tile([P, C], f32, tag="pa", name=f"p{tag}{tc_i}")
                pp = pa[:CF, :]
                for i2 in range(KC // 2):
                    nc.tensor.matmul(pp, w_sb[:, 2 * i2:2 * i2 + 2, :],
                                     xt[tc_i][:, 2 * i2:2 * i2 + 2, :],
                                     start=(i2 == 0), stop=(i2 == KC // 2 - 1),
                                     perf_mode=DR)
                sl = slice(tc_i * C, (tc_i + 1) * C)
                nc.vector.tensor_scalar_add(dst[:CF, sl], pp, b_sb)
                nc.gpsimd.dma_start(dst[CF:, sl], dst[:CF, sl])

            prep_s_exp(0)
            s0 = spair_queues[0]
            ns0 = 0

            # s0 pair jc2 uses f chunk jc2//2 and g chunk 0 -- woven
            # once their dup DMAs have landed
            for tc_i in range(NT):
                if tc_i < NKT:
                    proj_fg(tc_i, wf_sb, bf_sb, f_sb, "f")
                proj_fg(tc_i, wg_sb, bg_sb, g_sb, "g")
                if 2 <= tc_i < 2 + NJB // 4:     # pairs 0..3 here
                    s0[ns0]()
                    ns0 += 1

            for jb in range(NJB):
                tc_i, jl = divmod(jb, KC)
                ph = psA.tile([P, C], f32, tag="pa")
                for i2 in range(KC // 2):
                    nc.tensor.matmul(ph,
                                     xt[tc_i][:, 2 * i2:2 * i2 + 2,
                                              jl * P:(jl + 1) * P],
                                     wh_sb[:, 2 * i2:2 * i2 + 2, :],
                                     start=(i2 == 0), stop=(i2 == KC // 2 - 1),
                                     perf_mode=DR)
                if jb % 4 == 3:
                    nc.scalar.activation(h_sb[:, jb, :], ph, AFT.Copy)
                else:
                    nc.vector.tensor_copy(h_sb[:, jb, :], ph)
                # front-load the remaining s0 pairs (jb 1,3,5,7) so the
                # last exp0 lands well before phase B's first o-matmul
                if jb % 2 == 1 and ns0 < len(s0):
                    s0[ns0]()
                    ns0 += 1
            while ns0 < len(s0):
                s0[ns0]()
                ns0 += 1

        # ---- Phase B ----
        with tc.tile_pool(name="psO", bufs=2, space="PSUM") as psO, \
             tc.tile_pool(name="psR", bufs=2, space="PSUM") as psR:

            for sup in range(NSUP):
                if sup + 1 < NSUP:
                    prep_s_exp(sup + 1)
                snext = spair_queues.get(sup + 1, [])
                expT = expT_tiles.pop(sup)
                for q in range(C // P):
                    iq = sup * (C // P) + q
                    po = psO.tile([P, C], f32, tag="po")
                    pr = psR.tile([P, 1], f32, tag="pr")
                    for j in range(NJB // 2):
                        lhs = expT[:, q, 2 * j:2 * j + 2, :]
                        nc.tensor.matmul(po, lhs, h_sb[:, 2 * j:2 * j + 2, :],
                                         start=(j == 0),
                                         stop=(j == NJB // 2 - 1),
                                         perf_mode=DR)
                        nc.tensor.matmul(pr, lhs, ones2,
                                         start=(j == 0),
                                         stop=(j == NJB // 2 - 1),
                                         perf_mode=DR)
                        if j % 4 == 1:
                            slot = q * 2 + j // 4
                            if slot < len(snext):
                                snext[slot]()
                    nc.vector.tensor_copy(r_sb[:, iq:iq + 1], pr)
                    ot = fin.tile([P, C], f32, tag="ot")
                    if iq == NQT - 1:
                        # final store in halves: the first half's DMA
                        # overlaps the second half's eviction (tail)
                        for hf in range(2):
                            sl = slice(hf * (C // 2), (hf + 1) * (C // 2))
                            nc.vector.tensor_copy(ot[:, sl], po[:, sl])
                            nc.sync.dma_start(
                                po_out[iq * P:(iq + 1) * P, sl], ot[:, sl])
                    else:
                        nc.vector.tensor_copy(ot, po)
                        nc.sync.dma_start(po_out[iq * P:(iq + 1) * P, :], ot)
                # rowsums trickle out once per superblock from gpsimd
                nc.gpsimd.dma_start(
                    r_out[:, sup * (C // P):(sup + 1) * (C // P)],
                    r_sb[:, sup * (C // P):(sup + 1) * (C // P)])

    nc.compile()
    return nc


def _get_program() -> bass.Bass:
    global _PROGRAM
    if _PROGRAM is None:
        _PROGRAM = _build_program()
    return _PROGRAM


def kernel(x, kernel_f, kernel_g, kernel_h, bias_f, bias_g, bias_h, gamma,
           _trace=False, _trace_kwargs=None):
    global LAST_RESULTS
    x = np.asarray(x, np.float32)
    B = x.shape[0]
    xf = np.ascontiguousarray(x.reshape(B, N, C))
    gamma_f = np.asarray(gamma, np.float32).reshape(())

    e4 = ml_dtypes.float8_e4m3

    def pmajor(w, cout):
        w = np.asarray(w, np.float32).astype(e4)
        return np.ascontiguousarray(w.reshape(KC, P, cout).transpose(1, 0, 2))

    wf_np = pmajor(kernel_f, CF)
    wg_np = pmajor(kernel_g, CF)
    wh_np = pmajor(kernel_h, C)
    bf_np = np.ascontiguousarray(np.asarray(bias_f, np.float32).reshape(CF, 1))
    bg_np = np.ascontiguousarray(np.asarray(bias_g, np.float32).reshape(CF, 1))

    in_maps = []
    for c in range(8):
        b, half = divmod(c, 2)
        xT_full = xf[b].T                       # [C, N]
        if half == 0:
            xT_c = xT_full
        else:
            # rotate this core's key half to the front; g (queries) then
            # runs in rotated order and the host unrotates its po rows
            xT_c = np.concatenate([xT_full[:, NK:], xT_full[:, :NK]], axis=1)
        xT_c = xT_c.astype(e4).reshape(KC, P, NT, C).transpose(1, 2, 0, 3)
        in_maps.append({
            "xT": np.ascontiguousarray(xT_c),
            "wf": wf_np, "wg": wg_np, "wh": wh_np,
            "bfv": bf_np, "bgv": bg_np,
        })

    nc = _get_program()
    LAST_RESULTS = bass_utils.run_bass_kernel_spmd(
        nc, in_maps, core_ids=list(range(8)),
        trace=_trace, **(_trace_kwargs or {}))

    # host combine: exact flash-attention split-KV reduction (fixed
    # shift => partial numerators/denominators are directly addable);
    # h bias folded via softmax row-sum-1: beta@(h0+1*bh) = beta@h0+bh
    res_bias = (gamma_f * np.asarray(bias_h, np.float32)).reshape(1, C)
    result = np.empty((B, N, C), np.float32)
    for b in range(B):
        rs = LAST_RESULTS.results[2 * b]
        po = np.asarray(rs["po"])
        r = np.asarray(rs["r"]).T.reshape(N)
        rs2 = LAST_RESULTS.results[2 * b + 1]
        po2 = np.asarray(rs2["po"])
        r2 = np.asarray(rs2["r"]).T.reshape(N)
        # core 2b+1 ran queries in rotated token order
        po2 = np.concatenate([po2[NK:], po2[:NK]], axis=0)
        r2 = np.concatenate([r2[NK:], r2[:NK]], axis=0)
        o = (po + po2) / np.maximum(r + r2, 1e-30)[:, None]
        result[b] = gamma_f * o + xf[b] + res_bias
    return result.reshape(x.shape)


# revision 23
# speedup vs baseline: 1.0300x; 1.0061x over previous
"""Trainium2 Bass kernel for a SAGAN-style 2D attention layer.

Key-split (split-KV) variant: 8 cores = 4 batches x 2 KEY-halves.
Each core computes, for ALL 4096 queries of its batch, the partial
attention over ITS 2048 keys:

    po = exp(s_half - C_SHIFT) @ h_half     # [4096, 512]
    r  = rowsum(exp(s_half - C_SHIFT))      # [4096]

The host combines the two halves exactly (the fixed shift makes the
partial sums directly addable -- no per-row max renormalization):

    out = gamma * (poA + poB) / (rA + rB) + xf + gamma*bh

vs the query-split version this halves the per-core h projection
(values are only needed for the core's own keys) and shrinks phase A;
s / exp / o work per core is identical.

Other performance notes are inherited from the query-split version:
fp8 everywhere, DoubleRow o/h/f/g matmuls, per-512-token chunked
partition-major-contiguous DMAs, s-pairs row-tiled into PE quadrants
and woven 1-per-4-j into the o-loop so the scalar engine's EXP rate
(1114ns/pair) never throttles the PE, h-eviction casts split
vector/scalar. C_SHIFT = 104.5 per the host-simulated fp8 max logit
113.28 (8.8x fp8e5 overflow margin; softmax is shift-invariant so any
shift error cancels in the division, rows fully below the fp8e5
window flush to zero and are caught by the host's rowsum clamp).
"""

import ml_dtypes
import numpy as np
from contextlib import ExitStack

import concourse.bass as bass
import concourse.mybir as mybir
import concourse.tile as tile
from concourse import bacc, bass_utils

P = 128          # partitions
N = 4096         # tokens per batch (64*64)
NK = 2048        # keys per core (half)
C = 512          # channels
CF = 64          # f/g channels
KC = C // P      # contraction chunks over channels (4)
NJB = NK // P    # 16 local key blocks
NSUP = N // C    # 8 query super-blocks of 512
NQT = N // P     # 32 query blocks of 128
NT = N // C      # 8 token chunks of 512
NKT = NK // C    # 4 key-token chunks
C_SHIFT = 104.5

f32 = mybir.dt.float32
f8e4 = mybir.dt.float8e4
f8e5 = mybir.dt.float8e5

AFT = mybir.ActivationFunctionType
OP = mybir.AluOpType
DR = mybir.MatmulPerfMode.DoubleRow

_PROGRAM = None
LAST_RESULTS = None


def _build_program() -> bass.Bass:
    nc = bacc.Bacc("TRN2", target_bir_lowering=False, debug=False,
                   num_devices=8)

    xT = nc.dram_tensor("xT", [P, NT, KC, C], f8e4,
                        kind="ExternalInput").ap()
    wf = nc.dram_tensor("wf", [P, KC, CF], f8e4, kind="ExternalInput").ap()
    wg = nc.dram_tensor("wg", [P, KC, CF], f8e4, kind="ExternalInput").ap()
    wh = nc.dram_tensor("wh", [P, KC, C], f8e4, kind="ExternalInput").ap()
    bfv = nc.dram_tensor("bfv", [CF, 1], f32, kind="ExternalInput").ap()
    bgv = nc.dram_tensor("bgv", [CF, 1], f32, kind="ExternalInput").ap()
    po_out = nc.dram_tensor("po", [N, C], f32, kind="ExternalOutput").ap()
    r_out = nc.dram_tensor("r", [P, NQT], f32, kind="ExternalOutput").ap()

    with tile.TileContext(nc) as tc, ExitStack() as ctx:
        persist = ctx.enter_context(tc.tile_pool(name="persist", bufs=1))
        fin = ctx.enter_context(tc.tile_pool(name="fin", bufs=3))
        expp = ctx.enter_context(tc.tile_pool(name="expp", bufs=2))
        psS = ctx.enter_context(tc.tile_pool(name="psS", bufs=2, space="PSUM"))

        xt = []
        for tc_i in range(NT):
            xt.append(persist.tile([P, KC, C], f8e4, name=f"xt{tc_i}"))
        nc.sync.dma_start(xt[0], xT[:, 0, :, :])

        wf_sb = persist.tile([P, KC, CF], f8e4)
        nc.gpsimd.dma_start(wf_sb, wf)
        wg_sb = persist.tile([P, KC, CF], f8e4)
        nc.gpsimd.dma_start(wg_sb, wg)
        bf_sb = persist.tile([CF, 1], f32)
        nc.gpsimd.dma_start(bf_sb, bfv)
        bg_sb = persist.tile([CF, 1], f32)
        nc.gpsimd.dma_start(bg_sb, bgv)
        neg_shift = persist.tile([P, 1], f32)
        nc.vector.memset(neg_shift, -C_SHIFT)
        ones2 = persist.tile([P, 2, 1], f8e4)
        nc.vector.memset(ones2, 1.0)

        # tiny warm-up matmuls on ones2 while the first input DMAs are
        # in flight: ramps the PE pstate clock so the first real
        # matmuls (~11us) run at full frequency instead of ramping
        with tc.tile_pool(name="psW", bufs=1, space="PSUM") as psW:
            pw = psW.tile([1, 16], f32, tag="pw")
            for w in range(16):
                nc.tensor.matmul(pw[:, w:w + 1], ones2[:, 0, :],
                                 ones2[:, 0, :], start=True, stop=True)

        for tc_i in range(1, NT):
            nc.sync.dma_start(xt[tc_i], xT[:, tc_i, :, :])

        wh_sb = persist.tile([P, KC, C], f8e4)
        nc.sync.dma_start(wh_sb, wh)

        h_sb = persist.tile([P, NJB, C], f8e4)      # values, own keys
        f_sb = persist.tile([P, NK], f8e4)          # f^T own keys + dup
        g_sb = persist.tile([P, N], f8e4)           # g^T all queries + dup
        r_sb = persist.tile([P, NQT], f32)          # rowsums per q-block

        expT_tiles = {}
        spair_queues = {}

        def prep_s_exp(sup):
            expT = expp.tile([P, C // P, NJB, P], f8e5, tag="expT",
                             name=f"expT{sup}")
            expT_tiles[sup] = expT

            def mk(jc2):
                def emit():
                    jc = 2 * jc2
                    ps = psS.tile([P, 2, C], f32, tag="ps",
                                  name=f"ps{sup}_{jc2}")
                    nc.tensor.matmul(ps[:, 0, :],
                                     f_sb[:CF, jc * P:(jc + 1) * P],
                                     g_sb[:CF, sup * C:(sup + 1) * C],
                                     start=True, stop=True,
                                     tile_position=(0, 0))
                    nc.tensor.matmul(ps[:, 1, :],
                                     f_sb[CF:, (jc + 1) * P:(jc + 2) * P],
                                     g_sb[CF:, sup * C:(sup + 1) * C],
                                     start=True, stop=True,
                                     tile_position=(64, 0))
                    nc.scalar.activation(
                        expT[:, :, jc:jc + 2, :],
                        ps.rearrange("p two (qb col) -> p qb two col",
                                     qb=C // P),
                        AFT.Exp, bias=neg_shift)
                return emit
            spair_queues[sup] = [mk(j) for j in range(NJB // 2)]

        # ---- Phase A ----
        with tc.tile_pool(name="psA", bufs=4, space="PSUM") as psA:

            def proj_fg(tc_i, w_sb, b_sb, dst, tag):
                pa = psA.tile([P, C], f32, tag="pa", name=f"p{tag}{tc_i}")
                pp = pa[:CF, :]
                for i2 in range(KC // 2):
                    nc.tensor.matmul(pp, w_sb[:, 2 * i2:2 * i2 + 2, :],
                                     xt[tc_i][:, 2 * i2:2 * i2 + 2, :],
                                     start=(i2 == 0), stop=(i2 == KC // 2 - 1),
                                     perf_mode=DR)
                sl = slice(tc_i * C, (tc_i + 1) * C)
                nc.vector.tensor_scalar_add(dst[:CF, sl], pp, b_sb)
                nc.gpsimd.dma_start(dst[CF:, sl], dst[:CF, sl])

            prep_s_exp(0)
            s0 = spair_queues[0]
            ns0 = 0

            # s0 pair jc2 uses f chunk jc2//2 and g chunk 0 -- woven
            # once their dup DMAs have landed
            for tc_i in range(NT):
                if tc_i < NKT:
                    proj_fg(tc_i, wf_sb, bf_sb, f_sb, "f")
                proj_fg(tc_i, wg_sb, bg_sb, g_sb, "g")
                if 2 <= tc_i < 2 + NJB // 4:     # pairs 0..3 here
                    s0[ns0]()
                    ns0 += 1

            for jb in range(NJB):
                tc_i, jl = divmod(jb, KC)
                ph = psA.tile([P, C], f32, tag="pa")
                for i2 in range(KC // 2):
                    nc.tensor.matmul(ph,
                                     xt[tc_i][:, 2 * i2:2 * i2 + 2,
                                              jl * P:(jl + 1) * P],
                                     wh_sb[:, 2 * i2:2 * i2 + 2, :],
                                     start=(i2 == 0), stop=(i2 == KC // 2 - 1),
                                     perf_mode=DR)
                if jb % 4 == 3:
                    nc.scalar.activation(h_sb[:, jb, :], ph, AFT.Copy)
                else:
                    nc.vector.tensor_copy(h_sb[:, jb, :], ph)
                # front-load the remaining s0 pairs (jb 1,3,5,7) so the
                # last exp0 lands well before phase B's first o-matmul
                if jb % 2 == 1 and ns0 < len(s0):
                    s0[ns0]()
                    ns0 += 1
            while ns0 < len(s0):
                s0[ns0]()
                ns0 += 1

        # ---- Phase B ----
        with tc.tile_pool(name="psO", bufs=2, space="PSUM") as psO, \
             tc.tile_pool(name="psR", bufs=2, space="PSUM") as psR:

            for sup in range(NSUP):
                if sup + 1 < NSUP:
                    prep_s_exp(sup + 1)
                snext = spair_queues.get(sup + 1, [])
                expT = expT_tiles.pop(sup)
                for q in range(C // P):
                    iq = sup * (C // P) + q
                    po = psO.tile([P, C], f32, tag="po")
                    pr = psR.tile([P, 1], f32, tag="pr")
                    for j in range(NJB // 2):
                        lhs = expT[:, q, 2 * j:2 * j + 2, :]
                        nc.tensor.matmul(po, lhs, h_sb[:, 2 * j:2 * j + 2, :],
                                         start=(j == 0),
                                         stop=(j == NJB // 2 - 1),
                                         perf_mode=DR)
                        nc.tensor.matmul(pr, lhs, ones2,
                                         start=(j == 0),
                                         stop=(j == NJB // 2 - 1),
                                         perf_mode=DR)
                        if j % 4 == 1:
                            slot = q * 2 + j // 4
                            if slot < len(snext):
                                snext[slot]()
                    nc.vector.tensor_copy(r_sb[:, iq:iq + 1], pr)
                    ot = fin.tile([P, C], f32, tag="ot")
                    if iq == NQT - 1:
                        # final store in halves: the first half's DMA
                        # overlaps the second half's eviction (tail)
                        for hf in range(2):
                            sl = slice(hf * (C // 2), (hf + 1) * (C // 2))
                            nc.vector.tensor_copy(ot[:, sl], po[:, sl])
                            nc.sync.dma_start(
                                po_out[iq * P:(iq + 1) * P, sl], ot[:, sl])
                    else:
                        nc.vector.tensor_copy(ot, po)
                        nc.sync.dma_start(po_out[iq * P:(iq + 1) * P, :], ot)
                # rowsums trickle out once per superblock from gpsimd;
                # the last one goes via sync so the gpsimd teardown
                # DRAIN has no pending DMA to flush (~3us in the tail)
                eng = nc.sync if sup == NSUP - 1 else nc.gpsimd
                eng.dma_start(
                    r_out[:, sup * (C // P):(sup + 1) * (C // P)],
                    r_sb[:, sup * (C // P):(sup + 1) * (C // P)])

    nc.compile()
    return nc


def _get_program() -> bass.Bass:
    global _PROGRAM
    if _PROGRAM is None:
        _PROGRAM = _build_program()
    return _PROGRAM


def kernel(x, kernel_f, kernel_g, kernel_h, bias_f, bias_g, bias_h, gamma,
           _trace=False, _trace_kwargs=None):
    global LAST_RESULTS
    x = np.asarray(x, np.float32)
    B = x.shape[0]
    xf = np.ascontiguousarray(x.reshape(B, N, C))
    gamma_f = np.asarray(gamma, np.float32).reshape(())

    e4 = ml_dtypes.float8_e4m3

    def pmajor(w, cout):
        w = np.asarray(w, np.float32).astype(e4)
        return np.ascontiguousarray(w.reshape(KC, P, cout).transpose(1, 0, 2))

    wf_np = pmajor(kernel_f, CF)
    wg_np = pmajor(kernel_g, CF)
    wh_np = pmajor(kernel_h, C)
    bf_np = np.ascontiguousarray(np.asarray(bias_f, np.float32).reshape(CF, 1))
    bg_np = np.ascontiguousarray(np.asarray(bias_g, np.float32).reshape(CF, 1))

    in_maps = []
    for c in range(8):
        b, half = divmod(c, 2)
        xT_full = xf[b].T                       # [C, N]
        if half == 0:
            xT_c = xT_full
        else:
            # rotate this core's key half to the front; g (queries) then
            # runs in rotated order and the host unrotates its po rows
            xT_c = np.concatenate([xT_full[:, NK:], xT_full[:, :NK]], axis=1)
        xT_c = xT_c.astype(e4).reshape(KC, P, NT, C).transpose(1, 2, 0, 3)
        in_maps.append({
            "xT": np.ascontiguousarray(xT_c),
            "wf": wf_np, "wg": wg_np, "wh": wh_np,
            "bfv": bf_np, "bgv": bg_np,
        })

    nc = _get_program()
    LAST_RESULTS = bass_utils.run_bass_kernel_spmd(
        nc, in_maps, core_ids=list(range(8)),
        trace=_trace, **(_trace_kwargs or {}))

    # host combine: exact flash-attention split-KV reduction (fixed
    # shift => partial numerators/denominators are directly addable);
    # h bias folded via softmax row-sum-1: beta@(h0+1*bh) = beta@h0+bh
    res_bias = (gamma_f * np.asarray(bias_h, np.float32)).reshape(1, C)
    result = np.empty((B, N, C), np.float32)
    for b in range(B):
        rs = LAST_RESULTS.results[2 * b]
        po = np.asarray(rs["po"])
        r = np.asarray(rs["r"]).T.reshape(N)
        rs2 = LAST_RESULTS.results[2 * b + 1]
        po2 = np.asarray(rs2["po"])
        r2 = np.asarray(rs2["r"]).T.reshape(N)
        # core 2b+1 ran queries in rotated token order
        po2 = np.concatenate([po2[NK:], po2[:NK]], axis=0)
        r2 = np.concatenate([r2[NK:], r2[:NK]], axis=0)
        o = (po + po2) / np.maximum(r + r2, 1e-30)[:, None]
        result[b] = gamma_f * o + xf[b] + res_bias
    return result.reshape(x.shape)
